# revision 43
# baseline (speedup 1.0000x reference)
"""LoRA first-layer MLP kernel for 8 Trainium2 NeuronCores.

Computation:
    W_eff = W0 + 2.0 * (B @ A)            # [4096, 1024]
    h     = relu(x @ W_eff^T + b0)        # [16384, 4096]
    out   = (h @ W2^T + b2).squeeze(-1)   # [16384]

Sharding: data-parallel over batch; each of the 8 cores handles 2048 rows of
x and replicates the weights. No collectives needed.

Per-core device kernel (mixed fp8-DoubleRow / bf16 matmuls):
  - K split: d in [0,512) runs as 2 fp8e4 DoubleRow matmuls (K=256 each,
    2x PE throughput); d in [512,1024) runs as 4 bf16 matmuls. All six
    accumulate into one PSUM group per [128m, 512b] tile (6 instr vs 8
    for fp32r; measured end-to-end rel err ~1.7e-2 < 2e-2 gate).
  - W_eff^T built on device: W0^T streamed in bf16 (fp8-range pre-scaled
    by SW on host), LoRA rank-16 correction via PE matmul into PSUM,
    DVE add emits fp8 (d<512) or bf16 (d>=512) resident weight tiles.
  - W2/b0/relu fused into one ScalarE activation per tile:
    g = Relu(|w2|/S * psum + |w2|*b0) = |w2| * h. The m dimension is
    host-permuted so positive-w2 rows come first; per-chunk accumulators
    acc_pos/acc_neg collect g on VectorE/GpSimdE, and the final
    partition-reduce is ones^T @ acc_pos - ones^T @ acc_neg on the PE.
"""

import sys

sys.path.insert(0, "/opt/trn_rl_repo")

import numpy as np
import ml_dtypes

import concourse.bacc as bacc
import concourse.bass as bass
import concourse.mybir as mybir
import concourse.tile as tile
from concourse.bass_utils import run_bass_kernel_spmd

F32 = mybir.dt.float32
F32R = mybir.dt.float32r
BF16 = mybir.dt.bfloat16
FP8 = mybir.dt.float8e4

N_CORES = 8
B_FULL, D, M, R = 16384, 1024, 4096, 16
SCALING = 2.0
BS = B_FULL // N_CORES  # 2048 rows per core
NB = BS // 512  # 4 batch chunks per core
NM = M // 128  # 32 m-tiles
NM2 = M // 512  # 8 m-blocks of 512
KF8 = 512  # d-range [0, KF8) in fp8 DoubleRow
NP8 = KF8 // 256  # 2 DR pairs
NBF = (D - KF8) // 128  # 4 bf16 d-chunks
SX = 16.0  # x fp8 scale
SW = 1024.0  # W fp8 scale
S = SX * SW

_CACHE = {}


def _build_nc(n_pos):
    """n_pos: number of m rows with W2 >= 0 after the host permutation
    (positives first). Determines the acc_pos/acc_neg tile split."""
    full_pos_tiles = n_pos // 128  # tiles 0..full_pos_tiles-1 all-positive
    has_mix = (n_pos % 128) != 0
    mc_mix = full_pos_tiles if has_mix else -1
    # Last tile whose g feeds acc_p.
    mc_pos_done = mc_mix if has_mix else full_pos_tiles - 1

    nc = bacc.Bacc(
        "TRN2",
        target_bir_lowering=False,
        debug=False,
        num_devices=N_CORES,
    )
    xf8 = nc.dram_tensor("xf8", [NB, NP8, 128, 2 * 512], FP8, kind="ExternalInput").ap()
    xbf = nc.dram_tensor("xbf", [NB, 128, NBF * 512], BF16, kind="ExternalInput").ap()
    w08 = nc.dram_tensor("w08", [4, 128, M], BF16, kind="ExternalInput").ap()
    w0b = nc.dram_tensor("w0b", [NBF, 128, M], BF16, kind="ExternalInput").ap()
    a2p = nc.dram_tensor("a2p", [32, D], BF16, kind="ExternalInput").ap()
    btp = nc.dram_tensor("btp", [32, M], BF16, kind="ExternalInput").ap()
    ascd = nc.dram_tensor("ascd", [128, NM + 1], F32, kind="ExternalInput").ap()
    abid = nc.dram_tensor("abid", [128, NM + 1], F32, kind="ExternalInput").ap()
    b2s = nc.dram_tensor("b2s", [1, 1], F32, kind="ExternalInput").ap()
    onesd = nc.dram_tensor("ones", [128, 2], F32R, kind="ExternalInput").ap()
    onbd = nc.dram_tensor("onb", [128, 2], BF16, kind="ExternalInput").ap()
    out = nc.dram_tensor("out", [1, BS], F32, kind="ExternalOutput").ap()

    RELU = mybir.ActivationFunctionType.Relu
    DRM = mybir.MatmulPerfMode.DoubleRow

    with tile.TileContext(nc) as tc:
        with (
            tc.tile_pool(name="wp", bufs=1) as wp,
            tc.tile_pool(name="xp", bufs=2) as xp,
            tc.tile_pool(name="st", bufs=6) as stp,
            tc.tile_pool(name="hb", bufs=6) as hb,
            tc.tile_pool(name="ab", bufs=2) as ab,
            tc.tile_pool(name="cp", bufs=1) as cp,
            tc.tile_pool(name="psh", bufs=4, space="PSUM") as psh,
            tc.tile_pool(name="pso", bufs=1, space="PSUM") as pso,
            tc.tile_pool(name="psl", bufs=3, space="PSUM") as psl,
        ):
            # LoRA factors first (gate the block-0 LoRA matmuls): only R
            # meaningful rows; padding rows zeroed on-engine.
            A2 = cp.tile([128, D], BF16, tag="a2")
            nc.scalar.dma_start(out=A2[0:32, :], in_=a2p)
            BT = cp.tile([128, M], BF16, tag="bt")
            nc.scalar.dma_start(out=BT[0:32, :], in_=btp)
            for q, eng in enumerate((nc.vector, nc.gpsimd, nc.vector)):
                eng.memset(A2[32 + 32 * q : 64 + 32 * q, :], 0.0)
            for q, eng in enumerate((nc.gpsimd, nc.vector, nc.gpsimd)):
                eng.memset(BT[32 + 32 * q : 64 + 32 * q, :], 0.0)

            X8_0 = [
                xp.tile([128, 2, 512], FP8, tag=f"x8_{c}", name=f"x8_0_{c}")
                for c in range(NP8)
            ]
            XB_0 = xp.tile([128, NBF * 512], BF16, tag="xb", name="xb_0")

            # Resident W_eff^T: fp8 DR tiles per pair, bf16 tiles per d-chunk.
            W8 = [
                wp.tile([128, 2, M], FP8, tag=f"w8_{c}", name=f"w8_{c}")
                for c in range(NP8)
            ]
            WB = [
                wp.tile([128, M], BF16, tag=f"wb_{dc}", name=f"wb_{dc}")
                for dc in range(NBF)
            ]

            dma_engines = [nc.sync, nc.gpsimd, nc.scalar]
            prep_count = [0]

            def prep_step(mb, dc):
                """One W-prep step: DMA W0^T block, LoRA matmul, add->W tile."""
                sl = slice(mb * 512, (mb + 1) * 512)
                st = stp.tile([128, 512], BF16, tag="st")
                eng = dma_engines[prep_count[0] % 3]
                prep_count[0] += 1
                if dc < 4:
                    eng.dma_start(out=st[:], in_=w08[dc][:, sl])
                else:
                    eng.dma_start(out=st[:], in_=w0b[dc - 4][:, sl])
                lp = psl.tile([128, 512], F32, tag="lp")
                nc.tensor.matmul(
                    lp[:],
                    A2[:, dc * 128 : (dc + 1) * 128],
                    BT[:, sl],
                    start=True,
                    stop=True,
                )
                # GpSimd cannot read PSUM; LoRA adds stay on VectorE.
                if dc < 4:
                    c, i = dc // 2, dc % 2
                    nc.vector.tensor_add(W8[c][:, i, sl], st[:], lp[:])
                else:
                    nc.vector.tensor_add(WB[dc - 4][:, sl], st[:], lp[:])

            # Strict startup DMA priority: block-0 staging first (gates the
            # first compute tile), then x chunk 0, then constants, then
            # block 1. The rest is paced 2 steps per m-tile during chunk 0.
            for dc in range(8):
                prep_step(0, dc)

            for c in range(NP8):
                nc.sync.dma_start(out=X8_0[c][:], in_=xf8[0, c])
            nc.sync.dma_start(out=XB_0[:], in_=xbf[0])

            ASC = cp.tile([128, NM + 1], F32, tag="asc")
            nc.sync.dma_start(out=ASC[:], in_=ascd)
            ABI = cp.tile([128, NM + 1], F32, tag="abi")
            nc.sync.dma_start(out=ABI[:], in_=abid)
            B2 = cp.tile([1, 1], F32, tag="b2")
            nc.sync.dma_start(out=B2[:], in_=b2s)
            ONES = cp.tile([128, 2], F32R, tag="ones")
            nc.sync.dma_start(out=ONES[:], in_=onesd)
            ONB = cp.tile([128, 2], BF16, tag="onb")
            nc.sync.dma_start(out=ONB[:], in_=onbd)

            for dc in range(8):
                prep_step(1, dc)

            prep_queue = [(mb, dc) for mb in range(2, NM2) for dc in range(8)]
            prep_idx = [0]

            def pace_prep(n):
                for _ in range(n):
                    if prep_idx[0] < len(prep_queue):
                        prep_step(*prep_queue[prep_idx[0]])
                        prep_idx[0] += 1

            pending_reduce = []

            def emit_reduce(bc, acc_p, acc_n, have_p, have_n):
                op = pso.tile([1, 512], F32, tag="op")
                first = True
                if have_p:
                    nc.tensor.matmul(
                        op[:], ONES[:, 0:1], acc_p[:],
                        start=True, stop=not have_n,
                    )
                    first = False
                if have_n:
                    nc.tensor.matmul(
                        op[:], ONES[:, 1:2], acc_n[:],
                        start=first, stop=True,
                    )
                os_t = ab.tile([1, 512], F32, tag="os")
                nc.vector.tensor_scalar_add(os_t[:], op[:], B2[:, 0:1])
                nc.sync.dma_start(
                    out=out[:, bc * 512 : (bc + 1) * 512], in_=os_t[:]
                )

            # Fast tail: on the last chunk, the last two tiles' g feed the
            # output reduction directly (no acc add on the critical path).
            fast_tail = 0 < full_pos_tiles <= NM - 2 and (
                mc_mix < NM - 2 or mc_mix == -1
            )
            op_fin = None
            next_x = None

            for bc in range(NB):
                if bc == 0:
                    X8, XB = X8_0, XB_0
                else:
                    X8, XB = next_x
                final = bc == NB - 1
                acc_p = ab.tile([128, 512], F32R, tag="accp")
                acc_n = ab.tile([128, 512], F32R, tag="accn")
                have_p = have_n = False
                for mc in range(NM):
                    if bc == 0:
                        pace_prep(2)
                    if mc == 2 and pending_reduce:
                        emit_reduce(*pending_reduce.pop())
                    if mc == 4 and bc + 1 < NB:
                        nxt = bc + 1
                        X8n = [
                            xp.tile([128, 2, 512], FP8, tag=f"x8_{c}",
                                    name=f"x8_{nxt}_{c}")
                            for c in range(NP8)
                        ]
                        for c in range(NP8):
                            nc.sync.dma_start(out=X8n[c][:], in_=xf8[nxt, c])
                        XBn = xp.tile([128, NBF * 512], BF16, tag="xb",
                                      name=f"xb_{nxt}")
                        nc.sync.dma_start(out=XBn[:], in_=xbf[nxt])
                        next_x = (X8n, XBn)
                    if final and fast_tail and mc == NM - 2:
                        op_fin = pso.tile([1, 512], F32, tag="op")
                        nc.tensor.matmul(
                            op_fin[:], ONES[:, 0:1], acc_p[:],
                            start=True, stop=False,
                        )
                        nc.tensor.matmul(
                            op_fin[:], ONES[:, 1:2], acc_n[:],
                            start=False, stop=False,
                        )
                    hp = psh.tile([128, 512], F32, tag="hp")
                    for c in range(NP8):
                        nc.tensor.matmul(
                            hp[:],
                            W8[c][:, :, mc * 128 : (mc + 1) * 128],
                            X8[c][:],
                            start=(c == 0),
                            stop=False,
                            perf_mode=DRM,
                        )
                    for dc in range(NBF):
                        nc.tensor.matmul(
                            hp[:],
                            WB[dc][:, mc * 128 : (mc + 1) * 128],
                            XB[:, dc * 512 : (dc + 1) * 512],
                            start=False,
                            stop=(dc == NBF - 1),
                        )
                    # Chunk 0: VectorE is saturated by LoRA adds, alternate.
                    # Steady chunks: VectorE is idle and ~1.7x faster than
                    # GpSimd per op; give it every add.
                    if bc == 0:
                        veng = nc.vector if mc % 2 == 0 else nc.gpsimd
                    else:
                        veng = nc.vector

                    def emit_g(col, to_pos):
                        nonlocal have_p, have_n
                        g = hb.tile([128, 512], BF16, tag="g")
                        nc.scalar.activation(
                            g[:], hp[:], RELU,
                            bias=ABI[:, col : col + 1],
                            scale=ASC[:, col : col + 1],
                        )
                        acc = acc_p if to_pos else acc_n
                        if (have_p if to_pos else have_n):
                            veng.tensor_add(acc[:], acc[:], g[:])
                        else:
                            veng.tensor_copy(acc[:], g[:])
                        if to_pos:
                            have_p = True
                        else:
                            have_n = True

                    if final and fast_tail and mc >= NM - 2:
                        g = hb.tile([128, 512], BF16, tag="g")
                        nc.scalar.activation(
                            g[:], hp[:], RELU,
                            bias=ABI[:, mc : mc + 1],
                            scale=ASC[:, mc : mc + 1],
                        )
                        col = 0 if mc < full_pos_tiles else 1
                        nc.tensor.matmul(
                            op_fin[:], ONB[:, col : col + 1], g[:],
                            start=False, stop=(mc == NM - 1),
                        )
                    elif mc == mc_mix:
                        emit_g(mc, True)
                        emit_g(NM, False)
                    else:
                        emit_g(mc, mc < full_pos_tiles)
                if final and fast_tail:
                    os_t = ab.tile([1, 512], F32, tag="os")
                    nc.vector.tensor_scalar_add(os_t[:], op_fin[:], B2[:, 0:1])
                    nc.sync.dma_start(
                        out=out[:, bc * 512 : (bc + 1) * 512], in_=os_t[:]
                    )
                else:
                    pending_reduce.append((bc, acc_p, acc_n, have_p, have_n))
            while pending_reduce:
                emit_reduce(*pending_reduce.pop(0))

    nc.compile()
    return nc


def _prep_in_maps(x, W0, b0, A, B, W2, b2, perm, n_pos):
    e4m3 = ml_dtypes.float8_e4m3
    bf16 = ml_dtypes.bfloat16

    W0p = W0[perm]  # [M, D]
    Bp = B[perm]
    b0p = b0[perm]
    w2p = W2[0][perm]

    w0t = W0p.T  # [D, M]
    w08 = np.ascontiguousarray(
        (SW * w0t[:KF8]).reshape(4, 128, M)
    ).astype(bf16)
    w0b = np.ascontiguousarray(w0t[KF8:].reshape(NBF, 128, M)).astype(bf16)

    a2p = np.zeros((32, D), dtype=bf16)
    a2p[:R, :KF8] = (SCALING * SW * A[:, :KF8]).astype(bf16)
    a2p[:R, KF8:] = (SCALING * A[:, KF8:]).astype(bf16)
    btp = np.zeros((32, M), dtype=bf16)
    btp[:R] = Bp.T.astype(bf16)

    aw2 = np.abs(w2p)
    ascf = np.zeros((128, NM + 1), dtype=np.float32)
    abif = np.zeros((128, NM + 1), dtype=np.float32)
    ascf[:, :NM] = (aw2 / S).reshape(NM, 128).T
    abif[:, :NM] = (aw2 * b0p).reshape(NM, 128).T
    if n_pos % 128:
        mc_mix = n_pos // 128
        r = n_pos % 128  # rows [0, r) of tile mc_mix are positive
        # col NM: negative part of the mixed tile; col mc_mix: positive part
        ascf[:, NM] = ascf[:, mc_mix]
        abif[:, NM] = abif[:, mc_mix]
        ascf[r:, mc_mix] = 0.0
        abif[r:, mc_mix] = 0.0
        ascf[:r, NM] = 0.0
        abif[:r, NM] = 0.0

    b2v = b2.reshape(1, 1).astype(np.float32)
    ones = np.empty((128, 2), dtype=np.float32)
    ones[:, 0] = 1.0
    ones[:, 1] = -1.0
    onb = ones.astype(bf16)

    in_maps = []
    for c in range(N_CORES):
        xs = x[c * BS : (c + 1) * BS]  # [2048, 1024]
        # xf8[bc, pair, p, i, b] = e4m3(SX * xs[bc*512+b, pair*256+i*128+p])
        xf = (SX * xs[:, :KF8]).reshape(NB, 512, NP8, 2, 128)
        xf8 = np.ascontiguousarray(xf.transpose(0, 2, 4, 3, 1)).astype(e4m3)
        # xbf[bc, p, dc, b] = bf16(S * xs[bc*512+b, KF8+dc*128+p])
        xb = (S * xs[:, KF8:]).reshape(NB, 512, NBF, 128)
        xbf_a = np.ascontiguousarray(xb.transpose(0, 3, 2, 1)).astype(bf16)
        in_maps.append(
            {
                "xf8": xf8.reshape(NB, NP8, 128, 1024).view(np.uint8),
                "xbf": xbf_a.reshape(NB, 128, NBF * 512).view(np.uint16),
                "w08": w08.view(np.uint16),
                "w0b": w0b.view(np.uint16),
                "a2p": a2p.view(np.uint16),
                "btp": btp.view(np.uint16),
                "ascd": ascf,
                "abid": abif,
                "b2s": b2v,
                "ones": ones,
                "onb": onb.view(np.uint16),
            }
        )
    return in_maps


def kernel(x, W0, b0, A, B, W2, b2, _trace=False, _trace_kwargs=None):
    x = np.asarray(x, dtype=np.float32)
    W0 = np.asarray(W0, dtype=np.float32)
    b0 = np.asarray(b0, dtype=np.float32)
    A = np.asarray(A, dtype=np.float32)
    B = np.asarray(B, dtype=np.float32)
    W2 = np.asarray(W2, dtype=np.float32)
    b2 = np.asarray(b2, dtype=np.float32)

    # Stable permutation of m: positive-w2 rows first.
    w2row = W2[0]
    perm = np.argsort(w2row < 0, kind="stable")
    n_pos = int((w2row >= 0).sum())

    if _CACHE.get("n_pos") != n_pos:
        _CACHE["nc"] = _build_nc(n_pos)
        _CACHE["n_pos"] = n_pos
    nc = _CACHE["nc"]

    in_maps = _prep_in_maps(x, W0, b0, A, B, W2, b2, perm, n_pos)
    res = run_bass_kernel_spmd(
        nc,
        in_maps,
        list(range(N_CORES)),
        trace=_trace,
        **(_trace_kwargs or {}),
    )
    out = np.concatenate([r["out"].reshape(BS) for r in res.results])
    if _trace:
        _CACHE["last_results"] = res
    return out.astype(np.float32)


# revision 44
# speedup vs baseline: 1.0146x; 1.0146x over previous
"""LoRA first-layer MLP kernel for 8 Trainium2 NeuronCores.

Computation:
    W_eff = W0 + 2.0 * (B @ A)            # [4096, 1024]
    h     = relu(x @ W_eff^T + b0)        # [16384, 4096]
    out   = (h @ W2^T + b2).squeeze(-1)   # [16384]

Sharding: data-parallel over batch; each of the 8 cores handles 2048 rows of
x and replicates the weights. No collectives needed.

Per-core device kernel (mixed fp8-DoubleRow / bf16 matmuls):
  - K split: d in [0,512) runs as 2 fp8e4 DoubleRow matmuls (K=256 each,
    2x PE throughput); d in [512,1024) runs as 4 bf16 matmuls. All six
    accumulate into one PSUM group per [128m, 512b] tile (6 instr vs 8
    for fp32r; measured end-to-end rel err ~1.7e-2 < 2e-2 gate).
  - W_eff^T built on device: W0^T streamed in bf16 (fp8-range pre-scaled
    by SW on host), LoRA rank-16 correction via PE matmul into PSUM,
    DVE add emits fp8 (d<512) or bf16 (d>=512) resident weight tiles.
  - W2/b0/relu fused into one ScalarE activation per tile:
    g = Relu(|w2|/S * psum + |w2|*b0) = |w2| * h. The m dimension is
    host-permuted so positive-w2 rows come first; per-chunk accumulators
    acc_pos/acc_neg collect g on VectorE/GpSimdE, and the final
    partition-reduce is ones^T @ acc_pos - ones^T @ acc_neg on the PE.
"""

import sys

sys.path.insert(0, "/opt/trn_rl_repo")

import numpy as np
import ml_dtypes

import concourse.bacc as bacc
import concourse.bass as bass
import concourse.mybir as mybir
import concourse.tile as tile
from concourse.bass_utils import run_bass_kernel_spmd

F32 = mybir.dt.float32
F32R = mybir.dt.float32r
BF16 = mybir.dt.bfloat16
FP8 = mybir.dt.float8e4

N_CORES = 8
B_FULL, D, M, R = 16384, 1024, 4096, 16
SCALING = 2.0
BS = B_FULL // N_CORES  # 2048 rows per core
NB = BS // 512  # 4 batch chunks per core
NM = M // 128  # 32 m-tiles
NM2 = M // 512  # 8 m-blocks of 512
KF8 = 512  # d-range [0, KF8) in fp8 DoubleRow
NP8 = KF8 // 256  # 2 DR pairs
NBF = (D - KF8) // 128  # 4 bf16 d-chunks
SX = 16.0  # x fp8 scale
SW = 1024.0  # W fp8 scale
S = SX * SW

_CACHE = {}


def _build_nc(n_pos):
    """n_pos: number of m rows with W2 >= 0 after the host permutation
    (positives first). Determines the acc_pos/acc_neg tile split."""
    full_pos_tiles = n_pos // 128  # tiles 0..full_pos_tiles-1 all-positive
    has_mix = (n_pos % 128) != 0
    mc_mix = full_pos_tiles if has_mix else -1
    # Last tile whose g feeds acc_p.
    mc_pos_done = mc_mix if has_mix else full_pos_tiles - 1

    nc = bacc.Bacc(
        "TRN2",
        target_bir_lowering=False,
        debug=False,
        num_devices=N_CORES,
    )
    xf8 = nc.dram_tensor("xf8", [NB, NP8, 128, 2 * 512], FP8, kind="ExternalInput").ap()
    xbf = nc.dram_tensor("xbf", [NB, 128, NBF * 512], BF16, kind="ExternalInput").ap()
    w08 = nc.dram_tensor("w08", [4, 128, M], BF16, kind="ExternalInput").ap()
    w0b = nc.dram_tensor("w0b", [NBF, 128, M], BF16, kind="ExternalInput").ap()
    a2p = nc.dram_tensor("a2p", [32, D], BF16, kind="ExternalInput").ap()
    btp = nc.dram_tensor("btp", [32, M], BF16, kind="ExternalInput").ap()
    ascd = nc.dram_tensor("ascd", [128, NM + 1], F32, kind="ExternalInput").ap()
    abid = nc.dram_tensor("abid", [128, NM + 1], F32, kind="ExternalInput").ap()
    b2s = nc.dram_tensor("b2s", [1, 1], F32, kind="ExternalInput").ap()
    onesd = nc.dram_tensor("ones", [128, 2], F32R, kind="ExternalInput").ap()
    onbd = nc.dram_tensor("onb", [128, 2], BF16, kind="ExternalInput").ap()
    out = nc.dram_tensor("out", [1, BS], F32, kind="ExternalOutput").ap()

    RELU = mybir.ActivationFunctionType.Relu
    DRM = mybir.MatmulPerfMode.DoubleRow

    with tile.TileContext(nc) as tc:
        with (
            tc.tile_pool(name="wp", bufs=1) as wp,
            tc.tile_pool(name="xp", bufs=2) as xp,
            tc.tile_pool(name="st", bufs=6) as stp,
            tc.tile_pool(name="hb", bufs=6) as hb,
            tc.tile_pool(name="ab", bufs=2) as ab,
            tc.tile_pool(name="cp", bufs=1) as cp,
            tc.tile_pool(name="psh", bufs=4, space="PSUM") as psh,
            tc.tile_pool(name="pso", bufs=1, space="PSUM") as pso,
            tc.tile_pool(name="psl", bufs=3, space="PSUM") as psl,
        ):
            # LoRA factors first (gate the block-0 LoRA matmuls): only R
            # meaningful rows; padding rows zeroed on-engine.
            A2 = cp.tile([128, D], BF16, tag="a2")
            nc.scalar.dma_start(out=A2[0:32, :], in_=a2p)
            BT = cp.tile([128, M], BF16, tag="bt")
            nc.scalar.dma_start(out=BT[0:32, :], in_=btp)
            for q, eng in enumerate((nc.vector, nc.gpsimd, nc.vector)):
                eng.memset(A2[32 + 32 * q : 64 + 32 * q, :], 0.0)
            for q, eng in enumerate((nc.gpsimd, nc.vector, nc.gpsimd)):
                eng.memset(BT[32 + 32 * q : 64 + 32 * q, :], 0.0)

            X8_0 = [
                xp.tile([128, 2, 512], FP8, tag=f"x8_{c}", name=f"x8_0_{c}")
                for c in range(NP8)
            ]
            XB_0 = xp.tile([128, NBF * 512], BF16, tag="xb", name="xb_0")

            # Resident W_eff^T: fp8 DR tiles per pair, bf16 tiles per d-chunk.
            W8 = [
                wp.tile([128, 2, M], FP8, tag=f"w8_{c}", name=f"w8_{c}")
                for c in range(NP8)
            ]
            WB = [
                wp.tile([128, M], BF16, tag=f"wb_{dc}", name=f"wb_{dc}")
                for dc in range(NBF)
            ]

            dma_engines = [nc.sync, nc.gpsimd, nc.scalar]
            prep_count = [0]

            def prep_step(mb, dc):
                """One W-prep step: DMA W0^T block, LoRA matmul, add->W tile."""
                sl = slice(mb * 512, (mb + 1) * 512)
                st = stp.tile([128, 512], BF16, tag="st")
                eng = dma_engines[prep_count[0] % 3]
                prep_count[0] += 1
                if dc < 4:
                    eng.dma_start(out=st[:], in_=w08[dc][:, sl])
                else:
                    eng.dma_start(out=st[:], in_=w0b[dc - 4][:, sl])
                lp = psl.tile([128, 512], F32, tag="lp")
                nc.tensor.matmul(
                    lp[:],
                    A2[:, dc * 128 : (dc + 1) * 128],
                    BT[:, sl],
                    start=True,
                    stop=True,
                )
                # GpSimd cannot read PSUM; LoRA adds stay on VectorE.
                if dc < 4:
                    c, i = dc // 2, dc % 2
                    nc.vector.tensor_add(W8[c][:, i, sl], st[:], lp[:])
                else:
                    nc.vector.tensor_add(WB[dc - 4][:, sl], st[:], lp[:])

            # Strict startup DMA priority: block-0 staging first (gates the
            # first compute tile), then x chunk 0, then constants, then
            # block 1. The rest is paced 2 steps per m-tile during chunk 0.
            for dc in range(8):
                prep_step(0, dc)

            for c in range(NP8):
                nc.sync.dma_start(out=X8_0[c][:], in_=xf8[0, c])
            nc.sync.dma_start(out=XB_0[:], in_=xbf[0])

            ASC = cp.tile([128, NM + 1], F32, tag="asc")
            nc.sync.dma_start(out=ASC[:], in_=ascd)
            ABI = cp.tile([128, NM + 1], F32, tag="abi")
            nc.sync.dma_start(out=ABI[:], in_=abid)
            B2 = cp.tile([1, 1], F32, tag="b2")
            nc.sync.dma_start(out=B2[:], in_=b2s)
            ONES = cp.tile([128, 2], F32R, tag="ones")
            nc.sync.dma_start(out=ONES[:], in_=onesd)
            ONB = cp.tile([128, 2], BF16, tag="onb")
            nc.sync.dma_start(out=ONB[:], in_=onbd)

            for dc in range(8):
                prep_step(1, dc)

            prep_queue = [(mb, dc) for mb in range(2, NM2) for dc in range(8)]
            prep_idx = [0]

            def pace_prep(n):
                for _ in range(n):
                    if prep_idx[0] < len(prep_queue):
                        prep_step(*prep_queue[prep_idx[0]])
                        prep_idx[0] += 1

            pending_reduce = []

            def emit_reduce(bc, acc_p, acc_n, have_p, have_n):
                op = pso.tile([1, 512], F32, tag="op")
                first = True
                if have_p:
                    nc.tensor.matmul(
                        op[:], ONES[:, 0:1], acc_p[:],
                        start=True, stop=not have_n,
                    )
                    first = False
                if have_n:
                    nc.tensor.matmul(
                        op[:], ONES[:, 1:2], acc_n[:],
                        start=first, stop=True,
                    )
                os_t = ab.tile([1, 512], F32, tag="os")
                nc.vector.tensor_scalar_add(os_t[:], op[:], B2[:, 0:1])
                nc.sync.dma_start(
                    out=out[:, bc * 512 : (bc + 1) * 512], in_=os_t[:]
                )

            # Fast tail: on the last chunk, the last two tiles' g feed the
            # output reduction directly (no acc add on the critical path).
            fast_tail = 0 < full_pos_tiles <= NM - 2 and (
                mc_mix < NM - 2 or mc_mix == -1
            )
            op_fin = None
            next_x = None

            for bc in range(NB):
                if bc == 0:
                    X8, XB = X8_0, XB_0
                else:
                    X8, XB = next_x
                final = bc == NB - 1
                acc_p = ab.tile([128, 512], F32R, tag="accp")
                acc_n = ab.tile([128, 512], F32R, tag="accn")
                have_p = have_n = False
                for mc in range(NM):
                    if bc == 0:
                        pace_prep(2)
                    if mc == 2 and pending_reduce:
                        emit_reduce(*pending_reduce.pop())
                    if mc == 4 and bc + 1 < NB:
                        nxt = bc + 1
                        X8n = [
                            xp.tile([128, 2, 512], FP8, tag=f"x8_{c}",
                                    name=f"x8_{nxt}_{c}")
                            for c in range(NP8)
                        ]
                        for c in range(NP8):
                            nc.sync.dma_start(out=X8n[c][:], in_=xf8[nxt, c])
                        XBn = xp.tile([128, NBF * 512], BF16, tag="xb",
                                      name=f"xb_{nxt}")
                        nc.sync.dma_start(out=XBn[:], in_=xbf[nxt])
                        next_x = (X8n, XBn)
                    if final and fast_tail and mc == NM - 2:
                        op_fin = pso.tile([1, 512], F32, tag="op")
                        nc.tensor.matmul(
                            op_fin[:], ONES[:, 0:1], acc_p[:],
                            start=True, stop=False,
                        )
                        nc.tensor.matmul(
                            op_fin[:], ONES[:, 1:2], acc_n[:],
                            start=False, stop=False,
                        )
                    hp = psh.tile([128, 512], F32, tag="hp")
                    # Alternate DR-first / bf16-first per tile so consecutive
                    # tiles meet with the same PE dtype at the boundary
                    # (halves fp8<->bf16 mode switches).
                    seq = [("dr", c) for c in range(NP8)] + [
                        ("bf", dc) for dc in range(NBF)
                    ]
                    if mc % 2 == 1:
                        seq = seq[::-1]
                    for k, (kind, idx) in enumerate(seq):
                        if kind == "dr":
                            nc.tensor.matmul(
                                hp[:],
                                W8[idx][:, :, mc * 128 : (mc + 1) * 128],
                                X8[idx][:],
                                start=(k == 0),
                                stop=(k == len(seq) - 1),
                                perf_mode=DRM,
                            )
                        else:
                            nc.tensor.matmul(
                                hp[:],
                                WB[idx][:, mc * 128 : (mc + 1) * 128],
                                XB[:, idx * 512 : (idx + 1) * 512],
                                start=(k == 0),
                                stop=(k == len(seq) - 1),
                            )
                    # Chunk 0: VectorE is saturated by LoRA adds, alternate.
                    # Steady chunks: VectorE is idle and ~1.7x faster than
                    # GpSimd per op; give it every add.
                    if bc == 0:
                        veng = nc.vector if mc % 2 == 0 else nc.gpsimd
                    else:
                        veng = nc.vector

                    def emit_g(col, to_pos):
                        nonlocal have_p, have_n
                        g = hb.tile([128, 512], BF16, tag="g")
                        nc.scalar.activation(
                            g[:], hp[:], RELU,
                            bias=ABI[:, col : col + 1],
                            scale=ASC[:, col : col + 1],
                        )
                        acc = acc_p if to_pos else acc_n
                        if (have_p if to_pos else have_n):
                            veng.tensor_add(acc[:], acc[:], g[:])
                        else:
                            veng.tensor_copy(acc[:], g[:])
                        if to_pos:
                            have_p = True
                        else:
                            have_n = True

                    if final and fast_tail and mc >= NM - 2:
                        g = hb.tile([128, 512], BF16, tag="g")
                        nc.scalar.activation(
                            g[:], hp[:], RELU,
                            bias=ABI[:, mc : mc + 1],
                            scale=ASC[:, mc : mc + 1],
                        )
                        col = 0 if mc < full_pos_tiles else 1
                        nc.tensor.matmul(
                            op_fin[:], ONB[:, col : col + 1], g[:],
                            start=False, stop=(mc == NM - 1),
                        )
                    elif mc == mc_mix:
                        emit_g(mc, True)
                        emit_g(NM, False)
                    else:
                        emit_g(mc, mc < full_pos_tiles)
                if final and fast_tail:
                    os_t = ab.tile([1, 512], F32, tag="os")
                    nc.vector.tensor_scalar_add(os_t[:], op_fin[:], B2[:, 0:1])
                    nc.sync.dma_start(
                        out=out[:, bc * 512 : (bc + 1) * 512], in_=os_t[:]
                    )
                else:
                    pending_reduce.append((bc, acc_p, acc_n, have_p, have_n))
            while pending_reduce:
                emit_reduce(*pending_reduce.pop(0))

    nc.compile()
    return nc


def _prep_in_maps(x, W0, b0, A, B, W2, b2, perm, n_pos):
    e4m3 = ml_dtypes.float8_e4m3
    bf16 = ml_dtypes.bfloat16

    W0p = W0[perm]  # [M, D]
    Bp = B[perm]
    b0p = b0[perm]
    w2p = W2[0][perm]

    w0t = W0p.T  # [D, M]
    w08 = np.ascontiguousarray(
        (SW * w0t[:KF8]).reshape(4, 128, M)
    ).astype(bf16)
    w0b = np.ascontiguousarray(w0t[KF8:].reshape(NBF, 128, M)).astype(bf16)

    a2p = np.zeros((32, D), dtype=bf16)
    a2p[:R, :KF8] = (SCALING * SW * A[:, :KF8]).astype(bf16)
    a2p[:R, KF8:] = (SCALING * A[:, KF8:]).astype(bf16)
    btp = np.zeros((32, M), dtype=bf16)
    btp[:R] = Bp.T.astype(bf16)

    aw2 = np.abs(w2p)
    ascf = np.zeros((128, NM + 1), dtype=np.float32)
    abif = np.zeros((128, NM + 1), dtype=np.float32)
    ascf[:, :NM] = (aw2 / S).reshape(NM, 128).T
    abif[:, :NM] = (aw2 * b0p).reshape(NM, 128).T
    if n_pos % 128:
        mc_mix = n_pos // 128
        r = n_pos % 128  # rows [0, r) of tile mc_mix are positive
        # col NM: negative part of the mixed tile; col mc_mix: positive part
        ascf[:, NM] = ascf[:, mc_mix]
        abif[:, NM] = abif[:, mc_mix]
        ascf[r:, mc_mix] = 0.0
        abif[r:, mc_mix] = 0.0
        ascf[:r, NM] = 0.0
        abif[:r, NM] = 0.0

    b2v = b2.reshape(1, 1).astype(np.float32)
    ones = np.empty((128, 2), dtype=np.float32)
    ones[:, 0] = 1.0
    ones[:, 1] = -1.0
    onb = ones.astype(bf16)

    in_maps = []
    for c in range(N_CORES):
        xs = x[c * BS : (c + 1) * BS]  # [2048, 1024]
        # xf8[bc, pair, p, i, b] = e4m3(SX * xs[bc*512+b, pair*256+i*128+p])
        xf = (SX * xs[:, :KF8]).reshape(NB, 512, NP8, 2, 128)
        xf8 = np.ascontiguousarray(xf.transpose(0, 2, 4, 3, 1)).astype(e4m3)
        # xbf[bc, p, dc, b] = bf16(S * xs[bc*512+b, KF8+dc*128+p])
        xb = (S * xs[:, KF8:]).reshape(NB, 512, NBF, 128)
        xbf_a = np.ascontiguousarray(xb.transpose(0, 3, 2, 1)).astype(bf16)
        in_maps.append(
            {
                "xf8": xf8.reshape(NB, NP8, 128, 1024).view(np.uint8),
                "xbf": xbf_a.reshape(NB, 128, NBF * 512).view(np.uint16),
                "w08": w08.view(np.uint16),
                "w0b": w0b.view(np.uint16),
                "a2p": a2p.view(np.uint16),
                "btp": btp.view(np.uint16),
                "ascd": ascf,
                "abid": abif,
                "b2s": b2v,
                "ones": ones,
                "onb": onb.view(np.uint16),
            }
        )
    return in_maps


def kernel(x, W0, b0, A, B, W2, b2, _trace=False, _trace_kwargs=None):
    x = np.asarray(x, dtype=np.float32)
    W0 = np.asarray(W0, dtype=np.float32)
    b0 = np.asarray(b0, dtype=np.float32)
    A = np.asarray(A, dtype=np.float32)
    B = np.asarray(B, dtype=np.float32)
    W2 = np.asarray(W2, dtype=np.float32)
    b2 = np.asarray(b2, dtype=np.float32)

    # Stable permutation of m: positive-w2 rows first.
    w2row = W2[0]
    perm = np.argsort(w2row < 0, kind="stable")
    n_pos = int((w2row >= 0).sum())

    if _CACHE.get("n_pos") != n_pos:
        _CACHE["nc"] = _build_nc(n_pos)
        _CACHE["n_pos"] = n_pos
    nc = _CACHE["nc"]

    in_maps = _prep_in_maps(x, W0, b0, A, B, W2, b2, perm, n_pos)
    res = run_bass_kernel_spmd(
        nc,
        in_maps,
        list(range(N_CORES)),
        trace=_trace,
        **(_trace_kwargs or {}),
    )
    out = np.concatenate([r["out"].reshape(BS) for r in res.results])
    if _trace:
        _CACHE["last_results"] = res
    return out.astype(np.float32)
